# revision 9
# baseline (speedup 1.0000x reference)
"""Trainium2 Bass kernel for CombinedGeomAttention (wedge-norm attention).

reference math (ALPHA=1.0, EPS=1e-8, scale=1/8):
  D[l,s]   = q_l . k_s                      (per b,h)
  w2[l,s]  = relu(|q|^2 |k|^2 - D^2)
  scores   = sqrt(w2 + EPS) / 8
  A        = softmax_s(scores)
  V_out    = A @ V            -> [B, L, H, E]
  extra    = mean(|scores|)   (scores >= 0 here)

Sharding: B*H = 32 independent (b,h) attention problems, 4 per core on 8
NeuronCores. Full inputs in / full outputs out; this file shards and
gathers on host.

Device-side per (b,h), layout scores^T [s=partition, l=free]:
  D' = (K/8) @ Q^T  via f32r matmuls (KT8 weights [64,128], QT moving)
  sq = Square(D')   on ACT or DVE (copy+mult), split for engine balance
  m  = (Qn2B * kn2s_p) - sq   (scalar_tensor_tensor)  = w2/64, exact f32
  u  = Sqrt(m)      ACT, accum_out -> sum of scores   (= sqrt(w2)/8)
  e  = Exp(u)       ACT -> f32r
  O^T[d,l] (+denominator row) = sum_st Vhat_st^T @ e_st  (Vhat = [V | 1])
  out[l,d] = transpose(O^T) * (1/denom[l])
"""
import os
import sys
import math
import numpy as np

for _p in ('/opt/trn_rl_repo',):
    if os.path.isdir(_p) and _p not in sys.path:
        sys.path.append(_p)

import concourse.bass as bass
import concourse.mybir as mybir
import concourse.tile as tile
from concourse.bass_utils import run_bass_kernel_spmd
from concourse import masks
from concourse.vector_clock import ScopedClock
from concourse.tile_rust import add_dep_helper

F32 = mybir.dt.float32
F32R = mybir.dt.float32r
AF = mybir.ActivationFunctionType
ALU = mybir.AluOpType

B, L, S, H, E = 4, 2048, 2048, 8, 64
NCORES = 8
NBH = (B * H) // NCORES          # 4 (b,h) pairs per core
NST = S // 128                   # 16 s-tiles
NLB = 2                          # l halves of 1024
LB = L // NLB                    # 1024
NCH = 8                          # sqrt/exp chunks per l-half (2 s-tiles each)
# fraction of squares computed on DVE (copy+mult) instead of ACT, for balance
DVE_SQ_MOD = 2                   # st % DVE_SQ_MOD == 0 -> DVE path

_MAX_DRAIN_WAITS = 1


def _patched_drain_and_barrier(self, tick_clock, wait_clock):
    """This walrus build allows 1 sync-wait per Drain; split across drains."""
    drain_inst = self.nc.sync.drain()
    di = drain_inst.ins
    wait_clock.add_sem_waits(di, ScopedClock({None: tick_clock.global_clock}))
    si = di.sync_info
    waits = list(si.on_wait) if si else []
    if len(waits) > _MAX_DRAIN_WAITS:
        si.on_wait = waits[:_MAX_DRAIN_WAITS]
        rest = waits[_MAX_DRAIN_WAITS:]
        while rest:
            d2 = self.nc.sync.drain()
            d2.ins.sync_info = mybir.SyncInfo(
                on_wait=rest[:_MAX_DRAIN_WAITS], on_update=[])
            rest = rest[_MAX_DRAIN_WAITS:]
    self.nc.all_engine_barrier()
    popped = self.nc._tile_sem_poison_stack.pop()
    assert popped is self._sem_poison
    self.nc.clear_and_free_semaphores(list(self.sems.allocated().values()))
    self.nc.all_engine_barrier()


tile.TileContext._drain_and_barrier = _patched_drain_and_barrier

_wf = [0]


def _split_excess_waits(nc, max_waits=1):
    """Walrus here caps per-instruction sync-waits; spill extras onto
    same-engine NoOps placed immediately before (engine program order)."""
    for fn in nc.m.functions:
        for bb in fn.blocks:
            out = []
            changed = False
            for inst in bb.instructions:
                si = inst.sync_info
                waits = list(si.on_wait) if si else []
                if len(waits) > max_waits:
                    spill, keep = waits[:-max_waits], waits[-max_waits:]
                    for w in spill:
                        _wf[0] += 1
                        nop = mybir.InstNoOp(name=f"I-waitfix-{_wf[0]}")
                        nop.engine = inst.engine
                        nop.sync_info = mybir.SyncInfo(on_wait=[w], on_update=[])
                        out.append(nop)
                    si.on_wait = keep
                    changed = True
                out.append(inst)
            if changed:
                bb.instructions = out


def build_kernel():
    nc = bass.Bass()
    q_d = nc.declare_dram_parameter("q", [NBH, L, E], F32, isOutput=False)
    k_d = nc.declare_dram_parameter("k", [NBH, S, E], F32, isOutput=False)
    v_d = nc.declare_dram_parameter("v", [NBH, S, E], F32, isOutput=False)
    o_d = nc.declare_dram_parameter("o", [NBH, L, E], F32, isOutput=True)
    ss_d = nc.declare_dram_parameter("ssum", [128, NBH * NLB * NCH], F32,
                                     isOutput=True)
    row_scr = nc.dram_tensor("row_scr", [NBH, 1, L], F32)  # qn2 row staging

    act_chain = []  # sqrt/exp ACT ops, chained to pin table-set batching

    def chain(bi):
        if act_chain:
            add_dep_helper(bi.ins, act_chain[-1].ins, sync=False,
                           reason="act-table-batch")
        act_chain.append(bi)

    from contextlib import ExitStack
    with tile.TileContext(nc) as tc, ExitStack() as ctx:
        constp = ctx.enter_context(tc.tile_pool(name="const", bufs=1))
        natp = ctx.enter_context(tc.tile_pool(name="nat", bufs=2))
        wtp = ctx.enter_context(tc.tile_pool(name="wt", bufs=2))
        vhp = ctx.enter_context(tc.tile_pool(name="vh", bufs=2))
        qbp = ctx.enter_context(tc.tile_pool(name="qb", bufs=1))
        mstp = ctx.enter_context(tc.tile_pool(name="mst", bufs=4))
        ustp = ctx.enter_context(tc.tile_pool(name="ust", bufs=8))
        estp = ctx.enter_context(tc.tile_pool(name="est", bufs=3))
        sqp = ctx.enter_context(tc.tile_pool(name="sqp", bufs=1))
        sq2p = ctx.enter_context(tc.tile_pool(name="sq2p", bufs=2))
        qrowp = ctx.enter_context(tc.tile_pool(name="qrowp", bufs=1))
        smallp = ctx.enter_context(tc.tile_pool(name="small", bufs=2))
        epp = ctx.enter_context(tc.tile_pool(name="ep", bufs=2))
        avsbp = ctx.enter_context(tc.tile_pool(name="avsbp", bufs=1))
        dpsp = ctx.enter_context(tc.tile_pool(name="dps", bufs=3, space="PSUM"))
        avpsp = ctx.enter_context(tc.tile_pool(name="avps", bufs=1, space="PSUM"))
        if True:
            ident = constp.tile([128, 128], F32)
            masks.make_identity(nc, ident[:])
            ssum = constp.tile([128, NBH * NLB * NCH], F32)
            nc.vector.memset(ssum[:], 0.0)

            for bh in range(NBH):
                # ---------- prep ----------
                knat = natp.tile([128, NST, E], F32, tag="nat")
                nc.sync.dma_start(
                    out=knat[:],
                    in_=k_d[bh].rearrange("(t p) e -> p t e", p=128))
                qnat = natp.tile([128, NST, E], F32, tag="nat")
                nc.sync.dma_start(
                    out=qnat[:],
                    in_=q_d[bh].rearrange("(t p) e -> p t e", p=128))
                vnat = natp.tile([128, NST, E], F32, tag="nat")
                nc.sync.dma_start(
                    out=vnat[:],
                    in_=v_d[bh].rearrange("(t p) e -> p t e", p=128))

                # norms: kn2s = sum(k^2)/64 per s-tile col; qn2cols = sum(q^2)
                ksq = sqp.tile([128, NST, E], F32, tag="sq")
                nc.vector.tensor_tensor(ksq[:], knat[:], knat[:], ALU.mult)
                kn2s = smallp.tile([128, NST], F32, tag="kn2")
                nc.vector.tensor_reduce(
                    kn2s[:], ksq[:], mybir.AxisListType.X, ALU.add)
                nc.vector.tensor_scalar_mul(kn2s[:], kn2s[:], 1.0 / 64.0)

                qsq = sqp.tile([128, NST, E], F32, tag="sq")
                nc.vector.tensor_tensor(qsq[:], qnat[:], qnat[:], ALU.mult)
                qn2c = smallp.tile([128, NST], F32, tag="qn2")
                nc.vector.tensor_reduce(
                    qn2c[:], qsq[:], mybir.AxisListType.X, ALU.add)

                # qn2 broadcast tile: transpose cols -> row -> DRAM -> bcast
                qt_ps = dpsp.tile([16, 128], F32, tag="dps")
                nc.tensor.transpose(qt_ps[:], qn2c[:], ident[:])
                qn2t = smallp.tile([16, 128], F32, tag="qn2t")
                nc.vector.tensor_copy(qn2t[:], qt_ps[:])
                qrow = qrowp.tile([1, L], F32, tag="qrow")
                nc.sync.dma_start(out=qrow[:], in_=qn2t[:])
                nc.sync.dma_start(out=row_scr[bh], in_=qrow[:])
                qn2b = qbp.tile([128, L], F32)
                nc.sync.dma_start(
                    out=qn2b[:], in_=row_scr[bh].to_broadcast((128, L)))

                # K^T/8 and Q^T, f32r, via PE transposes (batched copyback)
                kt8 = wtp.tile([64, NST * 128], F32R, tag="kt8")
                qtd = wtp.tile([64, L], F32R, tag="qtd")
                for g in range(4):  # 4 transposes per psum bank
                    tp = dpsp.tile([64, 512], F32, tag="dps")
                    for pr in range(4):
                        st = g * 4 + pr
                        nc.tensor.transpose(
                            tp[:, pr * 128:(pr + 1) * 128],
                            knat[:, st, :], ident[:])
                    nc.vector.tensor_scalar_mul(
                        kt8[:, g * 512:(g + 1) * 512], tp[:], 0.125)
                for g in range(4):
                    tp = dpsp.tile([64, 512], F32, tag="dps")
                    for pr in range(4):
                        lt = g * 4 + pr
                        nc.tensor.transpose(
                            tp[:, pr * 128:(pr + 1) * 128],
                            qnat[:, lt, :], ident[:])
                    nc.vector.tensor_scalar_mul(
                        qtd[:, g * 512:(g + 1) * 512], tp[:], 1.0)

                # Vhat = [V | 1] in f32r
                vh = vhp.tile([128, NST, E + 1], F32R)
                nc.vector.tensor_scalar_mul(
                    vh[:, :, 0:E], vnat[:], 1.0)
                ones16 = smallp.tile([128, NST, 1], F32, tag="ones")
                nc.vector.memset(ones16[:], 1.0)
                nc.vector.tensor_scalar_mul(
                    vh[:, :, E:E + 1], ones16[:], 1.0)

                # ---------- main ----------
                for lb in range(NLB):
                    av = avpsp.tile([65, LB], F32)
                    e_tiles = {}
                    sqrt_ops = []
                    exp_srcs = []
                    for ch in range(NCH):
                        mstage = mstp.tile([128, 2, LB], F32, tag="mst")
                        for half in range(2):
                            st = 2 * ch + half
                            dps = dpsp.tile([128, LB], F32, tag="dps")
                            lhsT = kt8[:, st * 128:(st + 1) * 128]
                            rhs_t = qtd
                            for c in range(2):
                                nc.tensor.matmul(
                                    dps[:, c * 512:(c + 1) * 512], lhsT,
                                    rhs_t[:, lb * LB + c * 512:
                                          lb * LB + (c + 1) * 512],
                                    start=True, stop=True)
                            # square: split ACT / DVE(copy+mult)
                            sq_t = sq2p.tile([128, LB], F32, tag="sq2")
                            if st % DVE_SQ_MOD == 0:
                                cp = sq2p.tile([128, LB], F32, tag="cp")
                                nc.vector.tensor_copy(cp[:], dps[:])
                                nc.vector.tensor_tensor(
                                    sq_t[:], cp[:], cp[:], ALU.mult)
                            else:
                                nc.scalar.activation(sq_t[:], dps[:], AF.Square)
                            # m = qn2b*kn2s - sq  (= w2/64)
                            nc.vector.scalar_tensor_tensor(
                                mstage[:, half, :], qn2b[:, lb * LB:(lb + 1) * LB],
                                kn2s[:, st:st + 1], sq_t[:],
                                ALU.mult, ALU.subtract)
                        ustage = ustp.tile([128, 2, LB], F32, tag="ust")
                        col = (bh * NLB + lb) * NCH + ch
                        bi = nc.scalar.activation(
                            ustage[:], mstage[:], AF.Sqrt,
                            accum_out=ssum[:, col:col + 1])
                        sqrt_ops.append(bi)
                        exp_srcs.append(ustage)
                    for bi in sqrt_ops:
                        chain(bi)
                    for ch in range(NCH):
                        for half in range(2):
                            st = 2 * ch + half
                            estage = estp.tile([128, LB], F32R, tag="est")
                            bi = nc.scalar.activation(
                                estage[:], exp_srcs[ch][:, half, :], AF.Exp)
                            chain(bi)
                            for c in range(2):
                                nc.tensor.matmul(
                                    av[:, c * 512:(c + 1) * 512],
                                    vh[:, st, :],
                                    estage[:, c * 512:(c + 1) * 512],
                                    start=(st == 0), stop=(st == NST - 1))
                    # epilogue for this l-half
                    av_sb = avsbp.tile([65, LB], F32, tag="avsb")
                    nc.vector.tensor_copy(av_sb[:], av[:])
                    for c8 in range(LB // 128):
                        tpo = dpsp.tile([128, 65], F32, tag="dps")
                        nc.tensor.transpose(
                            tpo[:], av_sb[:, c8 * 128:(c8 + 1) * 128],
                            ident[0:65, 0:65])
                        rd = smallp.tile([128, 1], F32, tag="rd")
                        nc.vector.reciprocal(rd[:], tpo[:, 64:65])
                        ot = epp.tile([128, E], F32, tag="ot")
                        nc.vector.tensor_scalar(
                            ot[:], tpo[:, 0:E], rd[:], None, ALU.mult)
                        nc.sync.dma_start(
                            out=o_d[bh, lb * LB + c8 * 128:
                                    lb * LB + (c8 + 1) * 128, :],
                            in_=ot[:])
            nc.sync.dma_start(out=ss_d[:], in_=ssum[:])
    _split_excess_waits(nc)
    return nc


_NC_CACHE = None


def _get_nc():
    global _NC_CACHE
    if _NC_CACHE is None:
        _NC_CACHE = build_kernel()
    return _NC_CACHE


def kernel(queries, keys, values, trace=False):
    queries = np.asarray(queries, dtype=np.float32)
    keys = np.asarray(keys, dtype=np.float32)
    values = np.asarray(values, dtype=np.float32)
    nc = _get_nc()
    # shard: core c takes bh pairs [c*NBH, (c+1)*NBH); bh = b*H + h
    # per-core arrays [NBH, L, E] from [B, L, H, E]
    qs = np.ascontiguousarray(queries.transpose(0, 2, 1, 3).reshape(B * H, L, E))
    ks = np.ascontiguousarray(keys.transpose(0, 2, 1, 3).reshape(B * H, S, E))
    vs = np.ascontiguousarray(values.transpose(0, 2, 1, 3).reshape(B * H, S, E))
    in_maps = []
    for c in range(NCORES):
        sl = slice(c * NBH, (c + 1) * NBH)
        in_maps.append({"q": qs[sl], "k": ks[sl], "v": vs[sl]})
    res = run_bass_kernel_spmd(nc, in_maps, list(range(NCORES)), trace=trace)
    out = np.empty((B * H, L, E), np.float32)
    stot = 0.0
    for c in range(NCORES):
        out[c * NBH:(c + 1) * NBH] = res.results[c]["o"]
        stot += float(res.results[c]["ssum"].astype(np.float64).sum())
    V = out.reshape(B, H, L, E).transpose(0, 2, 1, 3)
    mean_scores = np.float32(stot / (B * H * L * S))
    if trace:
        return (np.ascontiguousarray(V), mean_scores), res
    return np.ascontiguousarray(V), mean_scores


# revision 11
# speedup vs baseline: 2418.0564x; 2418.0564x over previous
"""Trainium2 Bass kernel for CombinedGeomAttention (wedge-norm attention).

reference math (ALPHA=1.0, EPS=1e-8, scale=1/8):
  D[l,s]   = q_l . k_s                      (per b,h)
  w2[l,s]  = relu(|q|^2 |k|^2 - D^2)
  scores   = sqrt(w2 + EPS) / 8
  A        = softmax_s(scores)
  V_out    = A @ V            -> [B, L, H, E]
  extra    = mean(|scores|)   (scores >= 0 here)

Sharding: B*H = 32 independent (b,h) attention problems, 4 per core on 8
NeuronCores. Full inputs in / full outputs out; this file shards and
gathers on host.

Device-side per (b,h), layout scores^T [s=partition, l=free]:
  D' = (K/8) @ Q^T  via f32r matmuls (KT8 weights [64,128], QT moving)
  sq = Square(D')   on ACT or DVE (copy+mult), split for engine balance
  m  = (Qn2B * kn2s_p) - sq   (scalar_tensor_tensor)  = w2/64, exact f32
  u  = Sqrt(m)      ACT, accum_out -> sum of scores   (= sqrt(w2)/8)
  e  = Exp(u)       ACT -> f32r
  O^T[d,l] (+denominator row) = sum_st Vhat_st^T @ e_st  (Vhat = [V | 1])
  out[l,d] = transpose(O^T) * (1/denom[l])
"""
import os
import sys
import math
import numpy as np

for _p in ('/opt/trn_rl_repo',):
    if os.path.isdir(_p) and _p not in sys.path:
        sys.path.append(_p)

import concourse.bass as bass
import concourse.mybir as mybir
import concourse.tile as tile
from concourse.bass_utils import run_bass_kernel_spmd
from concourse import masks
from concourse.vector_clock import ScopedClock
from concourse.tile_rust import add_dep_helper

F32 = mybir.dt.float32
F32R = mybir.dt.float32r
AF = mybir.ActivationFunctionType
ALU = mybir.AluOpType

B, L, S, H, E = 4, 2048, 2048, 8, 64
NCORES = 8
NBH = (B * H) // NCORES          # 4 (b,h) pairs per core
NST = S // 128                   # 16 s-tiles
NLB = 2                          # l halves of 1024
LB = L // NLB                    # 1024
NCH = 8                          # sqrt/exp chunks per l-half (2 s-tiles each)
# fraction of squares computed on DVE (copy+mult) instead of ACT, for balance
DVE_SQ_MOD = 2                   # st % DVE_SQ_MOD == 0 -> DVE path

_MAX_DRAIN_WAITS = 1


def _patched_drain_and_barrier(self, tick_clock, wait_clock):
    """This walrus build allows 1 sync-wait per Drain; split across drains."""
    drain_inst = self.nc.sync.drain()
    di = drain_inst.ins
    wait_clock.add_sem_waits(di, ScopedClock({None: tick_clock.global_clock}))
    si = di.sync_info
    waits = list(si.on_wait) if si else []
    if len(waits) > _MAX_DRAIN_WAITS:
        si.on_wait = waits[:_MAX_DRAIN_WAITS]
        rest = waits[_MAX_DRAIN_WAITS:]
        while rest:
            d2 = self.nc.sync.drain()
            d2.ins.sync_info = mybir.SyncInfo(
                on_wait=rest[:_MAX_DRAIN_WAITS], on_update=[])
            rest = rest[_MAX_DRAIN_WAITS:]
    self.nc.all_engine_barrier()
    popped = self.nc._tile_sem_poison_stack.pop()
    assert popped is self._sem_poison
    self.nc.clear_and_free_semaphores(list(self.sems.allocated().values()))
    self.nc.all_engine_barrier()


tile.TileContext._drain_and_barrier = _patched_drain_and_barrier

_wf = [0]


def _split_excess_waits(nc, max_waits=1):
    """Walrus here caps per-instruction sync-waits; spill extras onto
    same-engine NoOps placed immediately before (engine program order)."""
    for fn in nc.m.functions:
        for bb in fn.blocks:
            out = []
            changed = False
            for inst in bb.instructions:
                si = inst.sync_info
                waits = list(si.on_wait) if si else []
                if len(waits) > max_waits:
                    spill, keep = waits[:-max_waits], waits[-max_waits:]
                    for w in spill:
                        _wf[0] += 1
                        nop = mybir.InstNoOp(name=f"I-waitfix-{_wf[0]}")
                        nop.engine = inst.engine
                        nop.sync_info = mybir.SyncInfo(on_wait=[w], on_update=[])
                        out.append(nop)
                    si.on_wait = keep
                    changed = True
                out.append(inst)
            if changed:
                bb.instructions = out


def build_kernel():
    nc = bass.Bass()
    q_d = nc.declare_dram_parameter("q", [NBH, L, E], F32, isOutput=False)
    k_d = nc.declare_dram_parameter("k", [NBH, S, E], F32, isOutput=False)
    v_d = nc.declare_dram_parameter("v", [NBH, S, E], F32, isOutput=False)
    o_d = nc.declare_dram_parameter("o", [NBH, L, E], F32, isOutput=True)
    ss_d = nc.declare_dram_parameter("ssum", [128, NBH * NLB * NCH], F32,
                                     isOutput=True)
    row_scr = nc.dram_tensor("row_scr", [NBH, 1, L], F32)  # qn2 row staging

    act_chain = []  # sqrt/exp ACT ops, chained to pin table-set batching

    def chain(bi):
        if act_chain:
            add_dep_helper(bi.ins, act_chain[-1].ins, sync=False,
                           reason="act-table-batch")
        act_chain.append(bi)

    from contextlib import ExitStack
    with tile.TileContext(nc) as tc, ExitStack() as ctx:
        constp = ctx.enter_context(tc.tile_pool(name="const", bufs=1))
        natp = ctx.enter_context(tc.tile_pool(name="nat", bufs=2))
        wtp = ctx.enter_context(tc.tile_pool(name="wt", bufs=2))
        vhp = ctx.enter_context(tc.tile_pool(name="vh", bufs=2))
        qbp = ctx.enter_context(tc.tile_pool(name="qb", bufs=1))
        mstp = ctx.enter_context(tc.tile_pool(name="mst", bufs=5))
        ustp = ctx.enter_context(tc.tile_pool(name="ust", bufs=8))
        estp = ctx.enter_context(tc.tile_pool(name="est", bufs=3))
        sqp = ctx.enter_context(tc.tile_pool(name="sqp", bufs=1))
        sq2p = ctx.enter_context(tc.tile_pool(name="sq2p", bufs=2))
        qrowp = ctx.enter_context(tc.tile_pool(name="qrowp", bufs=1))
        smallp = ctx.enter_context(tc.tile_pool(name="small", bufs=2))
        epp = ctx.enter_context(tc.tile_pool(name="ep", bufs=2))
        avsbp = ctx.enter_context(tc.tile_pool(name="avsbp", bufs=1))
        dpsp = ctx.enter_context(tc.tile_pool(name="dps", bufs=3, space="PSUM"))
        avpsp = ctx.enter_context(tc.tile_pool(name="avps", bufs=1, space="PSUM"))
        if True:
            ident = constp.tile([128, 128], F32)
            masks.make_identity(nc, ident[:])
            ssum = constp.tile([128, NBH * NLB * NCH], F32)
            nc.vector.memset(ssum[:], 0.0)

            for bh in range(NBH):
                # ---------- prep ----------
                knat = natp.tile([128, NST, E], F32, tag="nat")
                nc.gpsimd.dma_start(
                    out=knat[:],
                    in_=k_d[bh].rearrange("(t p) e -> p t e", p=128))
                qnat = natp.tile([128, NST, E], F32, tag="nat")
                nc.gpsimd.dma_start(
                    out=qnat[:],
                    in_=q_d[bh].rearrange("(t p) e -> p t e", p=128))
                vnat = natp.tile([128, NST, E], F32, tag="nat")
                nc.sync.dma_start(
                    out=vnat[:],
                    in_=v_d[bh].rearrange("(t p) e -> p t e", p=128))

                # norms: kn2s = sum(k^2)/64 per s-tile col; qn2cols = sum(q^2)
                ksq = sqp.tile([128, NST, E], F32, tag="sq")
                nc.vector.tensor_tensor(ksq[:], knat[:], knat[:], ALU.mult)
                kn2s = smallp.tile([128, NST], F32, tag="kn2")
                nc.vector.tensor_reduce(
                    kn2s[:], ksq[:], mybir.AxisListType.X, ALU.add)
                nc.vector.tensor_scalar_mul(kn2s[:], kn2s[:], 1.0 / 64.0)

                qsq = sqp.tile([128, NST, E], F32, tag="sq")
                nc.vector.tensor_tensor(qsq[:], qnat[:], qnat[:], ALU.mult)
                qn2c = smallp.tile([128, NST], F32, tag="qn2")
                nc.vector.tensor_reduce(
                    qn2c[:], qsq[:], mybir.AxisListType.X, ALU.add)

                # qn2 broadcast tile: transpose cols -> row -> DRAM -> bcast
                qt_ps = dpsp.tile([16, 128], F32, tag="dps")
                nc.tensor.transpose(qt_ps[:], qn2c[:], ident[:])
                qn2t = smallp.tile([16, 128], F32, tag="qn2t")
                nc.vector.tensor_copy(qn2t[:], qt_ps[:])
                qrow = qrowp.tile([1, L], F32, tag="qrow")
                nc.sync.dma_start(out=qrow[:], in_=qn2t[:])
                nc.sync.dma_start(out=row_scr[bh], in_=qrow[:])
                qn2b = qbp.tile([128, L], F32)
                nc.sync.dma_start(
                    out=qn2b[:], in_=row_scr[bh].to_broadcast((128, L)))

                # K^T/8 and Q^T, f32r, via PE transposes (batched copyback)
                kt8 = wtp.tile([64, NST * 128], F32R, tag="kt8")
                qtd = wtp.tile([64, L], F32R, tag="qtd")
                for g in range(4):  # 4 transposes per psum bank
                    tp = dpsp.tile([64, 512], F32, tag="dps")
                    for pr in range(4):
                        st = g * 4 + pr
                        nc.tensor.transpose(
                            tp[:, pr * 128:(pr + 1) * 128],
                            knat[:, st, :], ident[:])
                    nc.vector.tensor_scalar_mul(
                        kt8[:, g * 512:(g + 1) * 512], tp[:], 0.125)
                for g in range(4):
                    tp = dpsp.tile([64, 512], F32, tag="dps")
                    for pr in range(4):
                        lt = g * 4 + pr
                        nc.tensor.transpose(
                            tp[:, pr * 128:(pr + 1) * 128],
                            qnat[:, lt, :], ident[:])
                    nc.vector.tensor_scalar_mul(
                        qtd[:, g * 512:(g + 1) * 512], tp[:], 1.0)

                # Vhat = [V | 1] in f32r
                vh = vhp.tile([128, NST, E + 1], F32R)
                nc.vector.tensor_scalar_mul(
                    vh[:, :, 0:E], vnat[:], 1.0)
                ones16 = smallp.tile([128, NST, 1], F32, tag="ones")
                nc.vector.memset(ones16[:], 1.0)
                nc.vector.tensor_scalar_mul(
                    vh[:, :, E:E + 1], ones16[:], 1.0)

                # ---------- main ----------
                for lb in range(NLB):
                    av = avpsp.tile([65, LB], F32)
                    e_tiles = {}
                    sqrt_ops = []
                    exp_srcs = []
                    for ch in range(NCH):
                        mstage = mstp.tile([128, 2, LB], F32, tag="mst")
                        for half in range(2):
                            st = 2 * ch + half
                            dps = dpsp.tile([128, LB], F32, tag="dps")
                            lhsT = kt8[:, st * 128:(st + 1) * 128]
                            rhs_t = qtd
                            for c in range(2):
                                nc.tensor.matmul(
                                    dps[:, c * 512:(c + 1) * 512], lhsT,
                                    rhs_t[:, lb * LB + c * 512:
                                          lb * LB + (c + 1) * 512],
                                    start=True, stop=True)
                            # square: split ACT / DVE(copy+mult)
                            sq_t = sq2p.tile([128, LB], F32, tag="sq2")
                            if st % DVE_SQ_MOD == 0:
                                cp = sq2p.tile([128, LB], F32, tag="cp")
                                nc.vector.tensor_copy(cp[:], dps[:])
                                nc.vector.tensor_tensor(
                                    sq_t[:], cp[:], cp[:], ALU.mult)
                            else:
                                nc.scalar.activation(sq_t[:], dps[:], AF.Square)
                            # m = qn2b*kn2s - sq  (= w2/64)
                            nc.vector.scalar_tensor_tensor(
                                mstage[:, half, :], qn2b[:, lb * LB:(lb + 1) * LB],
                                kn2s[:, st:st + 1], sq_t[:],
                                ALU.mult, ALU.subtract)
                        ustage = ustp.tile([128, 2, LB], F32, tag="ust")
                        col = (bh * NLB + lb) * NCH + ch
                        bi = nc.scalar.activation(
                            ustage[:], mstage[:], AF.Sqrt,
                            accum_out=ssum[:, col:col + 1])
                        sqrt_ops.append(bi)
                        exp_srcs.append(ustage)
                    for bi in sqrt_ops:
                        chain(bi)
                    for ch in range(NCH):
                        for half in range(2):
                            st = 2 * ch + half
                            estage = estp.tile([128, LB], F32R, tag="est")
                            bi = nc.scalar.activation(
                                estage[:], exp_srcs[ch][:, half, :], AF.Exp)
                            chain(bi)
                            for c in range(2):
                                nc.tensor.matmul(
                                    av[:, c * 512:(c + 1) * 512],
                                    vh[:, st, :],
                                    estage[:, c * 512:(c + 1) * 512],
                                    start=(st == 0), stop=(st == NST - 1))
                    # epilogue for this l-half
                    av_sb = avsbp.tile([65, LB], F32, tag="avsb")
                    nc.vector.tensor_copy(av_sb[:], av[:])
                    for c8 in range(LB // 128):
                        tpo = dpsp.tile([128, 65], F32, tag="dps")
                        nc.tensor.transpose(
                            tpo[:], av_sb[:, c8 * 128:(c8 + 1) * 128],
                            ident[0:65, 0:65])
                        rd = smallp.tile([128, 1], F32, tag="rd")
                        nc.vector.reciprocal(rd[:], tpo[:, 64:65])
                        ot = epp.tile([128, E], F32, tag="ot")
                        nc.vector.tensor_scalar(
                            ot[:], tpo[:, 0:E], rd[:], None, ALU.mult)
                        nc.sync.dma_start(
                            out=o_d[bh, lb * LB + c8 * 128:
                                    lb * LB + (c8 + 1) * 128, :],
                            in_=ot[:])
            nc.sync.dma_start(out=ss_d[:], in_=ssum[:])
    _split_excess_waits(nc)
    return nc


_NC_CACHE = None


def _get_nc():
    global _NC_CACHE
    if _NC_CACHE is None:
        _NC_CACHE = build_kernel()
    return _NC_CACHE


def kernel(queries, keys, values, trace=False):
    queries = np.asarray(queries, dtype=np.float32)
    keys = np.asarray(keys, dtype=np.float32)
    values = np.asarray(values, dtype=np.float32)
    nc = _get_nc()
    # shard: core c takes bh pairs [c*NBH, (c+1)*NBH); bh = b*H + h
    # per-core arrays [NBH, L, E] from [B, L, H, E]
    qs = np.ascontiguousarray(queries.transpose(0, 2, 1, 3).reshape(B * H, L, E))
    ks = np.ascontiguousarray(keys.transpose(0, 2, 1, 3).reshape(B * H, S, E))
    vs = np.ascontiguousarray(values.transpose(0, 2, 1, 3).reshape(B * H, S, E))
    in_maps = []
    for c in range(NCORES):
        sl = slice(c * NBH, (c + 1) * NBH)
        in_maps.append({"q": qs[sl], "k": ks[sl], "v": vs[sl]})
    res = run_bass_kernel_spmd(nc, in_maps, list(range(NCORES)), trace=trace)
    out = np.empty((B * H, L, E), np.float32)
    stot = 0.0
    for c in range(NCORES):
        out[c * NBH:(c + 1) * NBH] = res.results[c]["o"]
        stot += float(res.results[c]["ssum"].astype(np.float64).sum())
    V = out.reshape(B, H, L, E).transpose(0, 2, 1, 3)
    mean_scores = np.float32(stot / (B * H * L * S))
    if trace:
        return (np.ascontiguousarray(V), mean_scores), res
    return np.ascontiguousarray(V), mean_scores
